# revision 25
# baseline (speedup 1.0000x reference)
"""GATv2 2-layer kernel for 8 Trainium2 NeuronCores (Bass/Tile, SPMD).

Strategy (per sharding hint): nodes sharded by id range across 8 cores;
edges partitioned by destination core and sorted by dst so the
segment-softmax/scatter-add becomes a PSUM-accumulated one-hot matmul
per 128-node destination block. Source features are exchanged via
AllGather of the per-shard linear transforms (xl tables), then fetched
per-edge with batched dma_gather (one SWDGE call per group of blocks).
Softmax runs without max-subtraction (scores are O(5)); normalization
is folded into a per-node divide after aggregation.

dma_gather uses int16 indices, so the 50000-row xl tables are addressed
with two complementary calls (rows < 32768 and >= 32768); each block's
edges are reordered so low-src / high-src edges occupy disjoint edge
tiles. The per-block tile schedule is uniform across cores so one SPMD
program serves all 8 cores.
"""
import sys
import numpy as np

sys.path.insert(0, '/opt/trn_rl_repo')

N_NODES = 50000
IN_CH = 128
HID = 32
HEADS = 4
C1 = HEADS * HID  # 128
OUT_CH = 64
SLOPE = 0.2
N_CORES = 8
SHARD = N_NODES // N_CORES          # 6250
NBLK = (SHARD + 127) // 128         # 49
LAST_VALID = SHARD - (NBLK - 1) * 128  # 106
PAD_LIDX = 300.0
GBLK = 4                            # blocks per gather group
HALF = SHARD // 2                   # shard-half split (A/B tables, int16-safe)


def _wrap16(vals):
    """dma_gather index layout: index j at [16k + j%16, j//16], k=0..7."""
    n = len(vals)
    arr = np.zeros((128, n // 16), np.int16)
    v = np.asarray(vals, np.int16).reshape(-1, 16)  # [n/16, 16]
    for k in range(8):
        arr[16 * k:16 * (k + 1), :] = v.T
    return arr


# ---------------------------------------------------------------- host side
def preprocess(edge_index):
    """Build the uniform per-core schedule with lo/hi src-split tiles.

    Group layout: [b0lo.. b1lo.. | b0hi.. b1hi..] per group of GBLK blocks.
    """
    ei = np.asarray(edge_index)
    loop = np.arange(N_NODES, dtype=ei.dtype)
    src = np.concatenate([ei[0], loop]).astype(np.int64)
    dst = np.concatenate([ei[1], loop]).astype(np.int64)
    order = np.argsort(dst, kind="stable")
    src, dst = src[order], dst[order]

    bounds = np.array([c * SHARD + min(b * 128, SHARD)
                       for c in range(N_CORES) for b in range(NBLK)] + [N_NODES],
                      dtype=np.int64)
    starts = np.searchsorted(dst, bounds)

    lo_e, hi_e = {}, {}
    cnt_lo = np.zeros((N_CORES, NBLK), np.int64)
    cnt_hi = np.zeros((N_CORES, NBLK), np.int64)
    for c in range(N_CORES):
        for b in range(NBLK):
            g = c * NBLK + b
            s = slice(starts[g], starts[g + 1])
            sb, db = src[s], dst[s]
            m = (sb % SHARD) < HALF
            lo_e[c, b] = (sb[m], db[m])
            hi_e[c, b] = (sb[~m], db[~m])
            cnt_lo[c, b] = int(m.sum())
            cnt_hi[c, b] = int((~m).sum())
    Tlo = -(-cnt_lo.max(axis=0) // 128)
    Thi = -(-cnt_hi.max(axis=0) // 128)

    groups = [(g0, min(g0 + GBLK, NBLK)) for g0 in range(0, NBLK, GBLK)]
    ntile = int(Tlo.sum() + Thi.sum())

    srcq = np.zeros((N_CORES, ntile * 128), np.int64)
    dstq = np.zeros((N_CORES, ntile * 128), np.int64)     # core-local dst row
    lidxq = np.full((N_CORES, ntile * 128), PAD_LIDX, np.float32)

    tile_of_block_lo, tile_of_block_hi = {}, {}
    pos = 0
    for (b0, b1) in groups:
        for b in range(b0, b1):
            tile_of_block_lo[b] = (pos, pos + int(Tlo[b]))
            pos += int(Tlo[b])
        for b in range(b0, b1):
            tile_of_block_hi[b] = (pos, pos + int(Thi[b]))
            pos += int(Thi[b])
    assert pos == ntile

    for c in range(N_CORES):
        for b in range(NBLK):
            for (t0, t1), (sb, db) in ((tile_of_block_lo[b], lo_e[c, b]),
                                       (tile_of_block_hi[b], hi_e[c, b])):
                n = len(sb)
                j = np.arange(n)
                flat = t0 * 128 + (j // 128) * 128 + (j % 128)
                srcq[c, flat] = sb
                dstq[c, flat] = db - c * SHARD
                lidxq[c, flat] = (db - c * SHARD - b * 128).astype(np.float32)

    return dict(Tlo=Tlo.astype(int), Thi=Thi.astype(int), groups=groups,
                ntile=ntile, tlo=tile_of_block_lo, thi=tile_of_block_hi,
                srcq=srcq, dstq=dstq, lidxq=lidxq)


def make_in_maps(x, W1l, W1r, att1, W2l, W2r, att2, sched):
    f16 = np.float16
    x = np.asarray(x)
    att1f = np.asarray(att1, np.float32).reshape(1, C1)
    att2f = np.asarray(att2, np.float32).reshape(1, OUT_CH)
    common = {
        "W1l": np.asarray(W1l, np.float32).astype(f16),
        "W1r": np.asarray(W1r, np.float32).astype(f16),
        "W2l": np.asarray(W2l, np.float32).astype(f16),
        "W2r": np.asarray(W2r, np.float32).astype(f16),
        "att1b": np.tile(att1f, (128, 1)).astype(f16),
        "att2b": np.tile(att2f, (128, 1)).astype(f16),
        "iotac": np.tile(np.arange(128, dtype=f16), (128, 1)),
        "ident": np.eye(128, dtype=f16),
    }
    xtf = np.ascontiguousarray(x.astype(f16).T)
    in_maps = []
    for c in range(N_CORES):
        srcq, dstq, lidxq = sched["srcq"][c], sched["dstq"][c], sched["lidxq"][c]
        sc, sr = srcq // SHARD, srcq % SHARD
        in_A = sr < HALF
        idx_a = np.where(in_A, sc * HALF + sr, 0)
        idx_b = np.maximum(sc * (SHARD - HALF) + (sr - HALF), 0)
        # host-built one-hot S tiles: S[p, t*128+m] = (lidx[t*128+p] == m)
        L = lidxq.reshape(-1, 128)
        S3 = (L[:, :, None] == np.arange(128)[None, None, :])
        Sq = np.ascontiguousarray(
            S3.transpose(1, 0, 2).reshape(128, -1).astype(f16))
        xs = x[c * SHARD:(c + 1) * SHARD].astype(f16)
        in_maps.append({**common,
                        "xTs": np.ascontiguousarray(xs.T),
                        "xTf": xtf,
                        "idxlo": _wrap16(idx_a),
                        "idxhi": _wrap16(idx_b),
                        "idxr": _wrap16(dstq),
                        "Sq": Sq,
                        })
    return in_maps


# ---------------------------------------------------------------- program
ABLATE = set()


def build_program(sched):
    n_cores, shard, nblk, last_valid = N_CORES, SHARD, NBLK, LAST_VALID
    n_nodes, c1, c2, heads = N_NODES, C1, OUT_CH, HEADS
    import concourse.bacc as bacc
    import concourse.mybir as mybir
    import concourse.tile as tile

    FP16 = mybir.dt.float16
    FP32 = mybir.dt.float32
    I16 = mybir.dt.int16
    AT = mybir.ActivationFunctionType
    ALU = mybir.AluOpType
    Tlo, Thi, groups = sched["Tlo"], sched["Thi"], sched["groups"]
    ntile = sched["ntile"]
    tlo, thi = sched["tlo"], sched["thi"]

    gt0, gtn = {}, {}
    for gi, (b0, b1) in enumerate(groups):
        t0 = tlo[b0][0]
        t1 = thi[b1 - 1][1]
        gt0[gi], gtn[gi] = t0, t1 - t0

    nc = bacc.Bacc("TRN2", target_bir_lowering=False, debug=False, num_devices=n_cores)

    xTs = nc.dram_tensor("xTs", [c1, shard], FP16, kind="ExternalInput")
    xTf = nc.dram_tensor("xTf", [c1, n_nodes], FP16, kind="ExternalInput")
    W1l = nc.dram_tensor("W1l", [c1, c1], FP16, kind="ExternalInput")
    W1r = nc.dram_tensor("W1r", [c1, c1], FP16, kind="ExternalInput")
    W2l = nc.dram_tensor("W2l", [c1, c2], FP16, kind="ExternalInput")
    W2r = nc.dram_tensor("W2r", [c1, c2], FP16, kind="ExternalInput")
    att1b = nc.dram_tensor("att1b", [128, c1], FP16, kind="ExternalInput")
    att2b = nc.dram_tensor("att2b", [128, c2], FP16, kind="ExternalInput")
    iotac = nc.dram_tensor("iotac", [128, 128], FP16, kind="ExternalInput")
    ident = nc.dram_tensor("ident", [128, 128], FP16, kind="ExternalInput")
    idxlo = nc.dram_tensor("idxlo", [128, ntile * 8], I16, kind="ExternalInput")
    idxhi = nc.dram_tensor("idxhi", [128, ntile * 8], I16, kind="ExternalInput")
    idxr = nc.dram_tensor("idxr", [128, ntile * 8], I16, kind="ExternalInput")
    Sq = nc.dram_tensor("Sq", [128, ntile * 128], FP16, kind="ExternalInput")
    out = nc.dram_tensor("out", [shard, c2], FP32, kind="ExternalOutput")

    with tile.TileContext(nc) as tc:
        with (
            tc.tile_pool(name="const", bufs=1) as cpool,
            tc.tile_pool(name="dram", bufs=1, space="DRAM") as dpool,
            tc.tile_pool(name="mm", bufs=2) as mpool,
            tc.tile_pool(name="idx", bufs=2) as ipool,
            tc.tile_pool(name="edge", bufs=2) as epool,
            tc.tile_pool(name="stile", bufs=2) as spool,
            tc.tile_pool(name="epi", bufs=2) as xpool,
            tc.tile_pool(name="ps", bufs=2, space="PSUM") as ppool,
            tc.tile_pool(name="ps2", bufs=2, space="PSUM") as p2pool,
            tc.tile_pool(name="ps4", bufs=2, space="PSUM") as p4pool,
            tc.tile_pool(name="ps3", bufs=2, space="PSUM") as p3pool,
        ):
            w1l_sb = cpool.tile([c1, c1], FP16, tag="w1l")
            w1r_sb = cpool.tile([c1, c1], FP16, tag="w1r")
            w2l_sb = cpool.tile([c1, c2], FP16, tag="w2l")
            w2r_sb = cpool.tile([c1, c2], FP16, tag="w2r")
            att1_sb = cpool.tile([128, c1], FP16, tag="att1")
            att2_sb = cpool.tile([128, c2], FP16, tag="att2")
            iota_sb = cpool.tile([128, 128], FP16, tag="iota")
            ident_sb = cpool.tile([128, 128], FP16, tag="ident")
            for sb_t, dr in ((w1l_sb, W1l), (w1r_sb, W1r), (w2l_sb, W2l), (w2r_sb, W2r),
                             (att1_sb, att1b), (att2_sb, att2b), (iota_sb, iotac), (ident_sb, ident)):
                nc.sync.dma_start(sb_t[:], dr[:])

            na = n_cores * (shard // 2)
            nb = n_nodes - na
            half = shard // 2
            xl1_A = dpool.tile([na, c1], FP16)
            xl1_B = dpool.tile([nb, c1], FP16)
            xr1_t = dpool.tile([shard, c1], FP16)
            xl2_shA = dpool.tile([half, c2], FP16)
            xl2_shB = dpool.tile([shard - half, c2], FP16)
            xl2A_ag = dpool.tile([na, c2], FP16)
            xl2B_ag = dpool.tile([nb, c2], FP16)
            xl2_A = dpool.tile([na, 128], FP16)   # padded rows for 256B gather
            xl2_B = dpool.tile([nb, 128], FP16)
            xr2_t = dpool.tile([shard, 128], FP16)     # padded rows

            # ---- P1a: full xl1 = x @ W1l on every core (no collective)
            def perm_pieces(r0, r1):
                """Split global row range [r0,r1) into maximal pieces that map
                contiguously into table A or B; yield (len, table_id, dst_row)."""
                r = r0
                while r < r1:
                    c, off = divmod(r, shard)
                    if off < half:
                        n = min(r1 - r, half - off)
                        yield n, 0, c * half + off
                    else:
                        n = min(r1 - r, shard - off)
                        yield n, 1, c * (shard - half) + (off - half)
                    r += n

            def mm_phase(src_dram, n_rows, w_sb, dsts, use_act_copy, perm=True):
                nblk_f = (n_rows + 127) // 128
                GP = 4      # blocks per psum group
                WB = 16     # blocks per write batch
                CHUNK = 8192  # xT columns per mega-load (64 blocks)
                for wb0 in range(0, nblk_f, WB):
                    wb1 = min(wb0 + WB, nblk_f)
                    sl = mpool.tile([128, WB, c1], FP16, tag="sl")
                    for g0 in range(wb0, wb1, GP):
                        g1 = min(g0 + GP, nblk_f)
                        if g0 * 128 % CHUNK == 0:
                            ch0 = g0 * 128
                            ncols = min(CHUNK, n_rows - ch0)
                            xt = mpool.tile([c1, CHUNK], FP16, tag="xt")
                            pad = -ncols % 128
                            if pad:
                                nc.vector.memset(xt[:, ncols:ncols + pad], 0.0)
                            nc.sync.dma_start(xt[:, :ncols], src_dram[:, ch0:ch0 + ncols])
                        ps = p4pool.tile([128, GP * 128], FP32, space="PSUM", tag="p1agg")
                        for b in range(g0, g1):
                            xoff = b * 128 - ch0
                            nc.tensor.matmul(out=ps[:, (b - g0) * c1:(b - g0 + 1) * c1],
                                             lhsT=xt[:, xoff:xoff + 128],
                                             rhs=w_sb[:], start=True, stop=True)
                        nbk = g1 - g0
                        so = g0 - wb0
                        h1 = nbk // 2
                        if h1:
                            nc.scalar.copy(
                                sl[:, so:so + h1, :].rearrange("p t c -> p (t c)"),
                                ps[:, 0:h1 * c1])
                        nc.vector.tensor_copy(
                            sl[:, so + h1:so + nbk, :].rearrange("p t c -> p (t c)"),
                            ps[:, h1 * c1:nbk * c1])
                    nr = min(128 * WB, n_rows - wb0 * 128)
                    pieces = (list(perm_pieces(wb0 * 128, wb0 * 128 + nr)) if perm
                              else [(nr, 0, wb0 * 128)])
                    pos = 0
                    for pi, (ln, tid, drow) in enumerate(pieces):
                        dst_dram = dsts[tid]
                        eng = nc.scalar if pi % 2 else nc.sync
                        q = 0
                        while q < ln:
                            t, p0 = divmod(pos + q, 128)
                            k = min(ln - q, 128 - p0)
                            if k == 128 and ln - q >= 128:
                                ntl = (ln - q) // 128
                                eng.dma_start(
                                    dst_dram[drow + q:drow + q + ntl * 128, :]
                                    .rearrange("(t p) c -> p t c", p=128),
                                    sl[:, t:t + ntl, :])
                                q += ntl * 128
                                continue
                            eng.dma_start(
                                dst_dram[drow + q:drow + q + k, :],
                                sl[p0:p0 + k, t, :])
                            q += k
                        pos += ln

            mm_phase(xTf, n_nodes, w1l_sb, (xl1_A, xl1_B), False)
            mm_phase(xTs, shard, w1r_sb, (xr1_t,), True, perm=False)

            def edge_layer(ch, cw, xl_tables, xr_table, att_sb, is_l1, after_block=None):
                nh = heads if is_l1 else 1
                hch = ch // nh
                for gi, (b0, b1) in enumerate(groups):
                    t0, tn = gt0[gi], gtn[gi]
                    ne = tn * 128
                    Ssb = spool.tile([128, tn, 128], FP16, tag="Ssb")
                    nc.sync.dma_start(Ssb[:].rearrange("p t m -> p (t m)"),
                                      Sq[:, t0 * 128:(t0 + tn) * 128])
                    ilo = ipool.tile([128, tn * 8], I16, tag="ilo")
                    ihi = ipool.tile([128, tn * 8], I16, tag="ihi")
                    ir = ipool.tile([128, tn * 8], I16, tag="ir")
                    nc.sync.dma_start(ilo[:], idxlo[:, t0 * 8:(t0 + tn) * 8])
                    nc.sync.dma_start(ihi[:], idxhi[:, t0 * 8:(t0 + tn) * 8])
                    nc.sync.dma_start(ir[:], idxr[:, t0 * 8:(t0 + tn) * 8])
                    xe = epool.tile([128, tn, cw], FP16, tag="xe")
                    xr = epool.tile([128, tn, cw], FP16, tag="xr")
                    n_lo = sum(int(Tlo[b]) for b in range(b0, b1))
                    n_hi = tn - n_lo
                    MAXT = 8  # 1024 descriptors per SWDGE call

                    def chunked_gather(dst, tbl, idxs, ta, tb):
                        for q0 in range(ta, tb, MAXT):
                            q1 = min(q0 + MAXT, tb)
                            nc.gpsimd.dma_gather(
                                out_ap=dst[:, q0:q1, :], in_ap=tbl,
                                idxs_ap=idxs[:, q0 * 8:q1 * 8],
                                num_idxs=(q1 - q0) * 128,
                                num_idxs_reg=(q1 - q0) * 128, elem_size=cw)

                    if n_lo:
                        chunked_gather(xe, xl_tables[0][:], ilo, 0, n_lo)
                    if n_hi:
                        chunked_gather(xe, xl_tables[1][:], ihi, n_lo, tn)
                    chunked_gather(xr, xr_table[:], ir, 0, tn)
                    # per-segment (lo/hi) chains so compute overlaps gathers
                    segs = []
                    if n_lo:
                        segs.append((0, n_lo))
                    if n_hi:
                        segs.append((n_lo, tn))
                    V = epool.tile([128, tn, ch + nh], FP16, tag="V")
                    p = spool.tile([128, tn * nh], FP16, tag="p")
                    for (sa, sb_) in segs:
                        sn = sb_ - sa
                        z = epool.tile([128, sn, ch], FP16, tag="z")
                        nc.vector.tensor_tensor(out=z[:], in0=xe[:, sa:sb_, 0:ch],
                                                in1=xr[:, sa:sb_, 0:ch], op=ALU.add)
                        nc.scalar.activation(z[:], z[:], AT.Prelu, alpha=SLOPE)
                        nc.vector.tensor_tensor(
                            out=z[:], in0=z[:],
                            in1=att_sb[:, :].unsqueeze(1).broadcast_to([128, sn, ch]),
                            op=ALU.mult)
                        score = spool.tile([128, sn * nh], FP32, tag="score")
                        nc.vector.tensor_reduce(
                            out=score[:], in_=z[:].rearrange("p t (h c) -> p (t h) c", h=nh),
                            axis=mybir.AxisListType.X, op=ALU.add)
                        nc.scalar.activation(p[:, sa * nh:sb_ * nh], score[:], AT.Exp)
                        nc.vector.tensor_tensor(
                            out=V[:, sa:sb_, 0:ch].rearrange("p t (h c) -> p t h c", h=nh),
                            in0=xe[:, sa:sb_, 0:ch].rearrange("p t (h c) -> p t h c", h=nh),
                            in1=p[:, sa * nh:sb_ * nh].rearrange("p (t h) -> p t h", h=nh)
                                .unsqueeze(3).broadcast_to([128, sn, nh, hch]),
                            op=ALU.mult)
                        nc.vector.tensor_copy(
                            V[:, sa:sb_, ch:ch + nh],
                            p[:, sa * nh:sb_ * nh].rearrange("p (t h) -> p t h", h=nh))
                    # per-block aggregation + epilogue
                    for b in range(b0, b1):
                        nt_valid = 128 if b < nblk - 1 else last_valid
                        tranges = [(tlo[b][0] - t0, tlo[b][1] - t0),
                                   (thi[b][0] - t0, thi[b][1] - t0)]
                        tiles = [t for (a, z2) in tranges for t in range(a, z2)]
                        psum = ppool.tile([128, ch + nh], FP32, space="PSUM", tag="agg")
                        for i, t in enumerate(tiles):
                            nc.tensor.matmul(out=psum[:], lhsT=Ssb[:, t, :], rhs=V[:, t, :],
                                             start=(i == 0), stop=(i == len(tiles) - 1))
                        dn = xpool.tile([128, nh], FP32, tag="dn")
                        nc.vector.tensor_scalar(out=dn[:], in0=psum[:, ch:ch + nh],
                                                scalar1=1e-16, scalar2=None, op0=ALU.add)
                        rd = xpool.tile([128, nh], FP32, tag="rd")
                        nc.vector.reciprocal(rd[:], dn[:])
                        ob = xpool.tile([128, ch], FP32, tag="ob")
                        nc.vector.tensor_tensor(
                            out=ob[:].rearrange("p (h c) -> p h c", h=nh),
                            in0=psum[:, 0:ch].rearrange("p (h c) -> p h c", h=nh),
                            in1=rd[:].unsqueeze(2).broadcast_to([128, nh, hch]),
                            op=ALU.mult)
                        if is_l1:
                            ei = xpool.tile([128, ch], FP32, tag="ei")
                            nc.vector.tensor_scalar(out=ei[:], in0=ob[:], scalar1=0.0,
                                                    scalar2=None, op0=ALU.min)
                            ex = xpool.tile([128, ch], FP32, tag="ex")
                            nc.scalar.activation(ex[:], ei[:], AT.Exp)
                            rm = xpool.tile([128, ch], FP32, tag="rm")
                            nc.vector.tensor_scalar(out=rm[:], in0=ob[:], scalar1=0.0,
                                                    scalar2=-1.0, op0=ALU.max, op1=ALU.add)
                            hb = xpool.tile([128, ch], FP16, tag="hb")
                            nc.vector.tensor_tensor(out=hb[:], in0=ex[:], in1=rm[:], op=ALU.add)
                            hT_ps = p3pool.tile([128, 128], FP16, space="PSUM", tag="hT")
                            nc.tensor.transpose(out=hT_ps[:], in_=hb[:], identity=ident_sb[:])
                            hT = xpool.tile([128, 128], FP16, tag="hTs")
                            nc.vector.tensor_copy(hT[:], hT_ps[:])
                            ps_a = p2pool.tile([128, c2], FP32, space="PSUM", tag="aux")
                            ps_b = p2pool.tile([128, c2], FP32, space="PSUM", tag="aux")
                            nc.tensor.matmul(out=ps_a[:], lhsT=hT[:], rhs=w2l_sb[:], start=True, stop=True)
                            nc.tensor.matmul(out=ps_b[:], lhsT=hT[:], rhs=w2r_sb[:], start=True, stop=True)
                            xa = xpool.tile([128, c2], FP16, tag="xa")
                            xb = xpool.tile([128, c2], FP16, tag="xb")
                            nc.vector.tensor_copy(xa[:], ps_a[:])
                            nc.scalar.copy(xb[:], ps_b[:])
                            r0 = b * 128
                            if r0 + nt_valid <= half:
                                nc.sync.dma_start(xl2_shA[r0:r0 + nt_valid, :], xa[:nt_valid, :])
                            elif r0 >= half:
                                nc.sync.dma_start(xl2_shB[r0 - half:r0 - half + nt_valid, :],
                                                  xa[:nt_valid, :])
                            else:
                                k = half - r0
                                nc.sync.dma_start(xl2_shA[r0:half, :], xa[:k, :])
                                nc.sync.dma_start(xl2_shB[0:r0 + nt_valid - half, :],
                                                  xa[k:nt_valid, :])
                            nc.sync.dma_start(xr2_t[b * 128:b * 128 + nt_valid, 0:c2], xb[:nt_valid, :])
                        else:
                            nc.sync.dma_start(out[b * 128:b * 128 + nt_valid, :], ob[:nt_valid, :])
                        if after_block is not None and b in after_block:
                            after_block[b]()

            # ---- P3: layer-1 edges; AG2a fires as soon as the A-half of
            # xl2_sh is written (mid-L1), AG2b after the last block.
            blkA = (half - 1) // 128          # last block writing rows < half

            def fire_ag2a():
                nc.gpsimd.collective_compute(
                    "AllGather", mybir.AluOpType.bypass,
                    replica_groups=[list(range(n_cores))],
                    ins=[xl2_shA.opt()], outs=[xl2A_ag.opt()],
                )
                nc.sync.dma_start(xl2_A[:, 0:c2], xl2A_ag[:])

            def fire_ag2b():
                nc.gpsimd.collective_compute(
                    "AllGather", mybir.AluOpType.bypass,
                    replica_groups=[list(range(n_cores))],
                    ins=[xl2_shB.opt()], outs=[xl2B_ag.opt()],
                )
                nc.sync.dma_start(xl2_B[:, 0:c2], xl2B_ag[:])

            edge_layer(c1, c1, (xl1_A, xl1_B), xr1_t, att1_sb, True,
                       after_block={blkA: fire_ag2a, nblk - 1: fire_ag2b})
            # ---- P5: layer-2 edges
            edge_layer(c2, 128, (xl2_A, xl2_B), xr2_t, att2_sb, False)

    nc.compile()
    return nc


_CACHE = {}


def _get_program(sched):
    key = (tuple(sched["Tlo"]), tuple(sched["Thi"]))
    if key not in _CACHE:
        _CACHE[key] = build_program(sched)
    return _CACHE[key]


def kernel(x, edge_index, W1l, W1r, att1, b1, W2l, W2r, att2, b2):
    from concourse.bass_utils import run_bass_kernel_spmd

    sched = preprocess(edge_index)
    nc = _get_program(sched)
    in_maps = make_in_maps(x, W1l, W1r, att1, W2l, W2r, att2, sched)
    res = run_bass_kernel_spmd(nc, in_maps, list(range(N_CORES)))
    o = np.concatenate([res.results[c]["out"] for c in range(N_CORES)], axis=0)
    o = o + np.asarray(b2, np.float32)[None, :]
    return o.astype(np.float32)
